# revision 18
# baseline (speedup 1.0000x reference)
"""Trainium2 Bass kernel for the AttentiveModule problem.

Reference computation (per batch element b, S=1024, D=512):
    att   = aspect @ inp.T / sqrt(len)                # [S,S]
    exp   = att * mask[:, None]                       # row mask (query dim)
    att_n = exp / (exp.sum(-1, keepdims=True) + 1e-4) # linear normalize
    w     = att_n @ inp                               # [S,D]
    ffn_inp = w + (inp + aspect) * mask[:, None]
    o1    = relu(ffn_inp @ w1.T + b1)
    o2    = relu(o1 @ w2.T + b2)
    final = 2*ffn_inp + o2
    out   = final / ||final||_2(axis=-1)

Sharding: data-parallel over batch, one batch element per NeuronCore
(8 cores).  Host prep is O(B*S*D), trivial vs the O(S*D^2) device work.

Algorithm (per core):
  - The row mask and linear normalization factor into a per-row scalar
    g[s] = mask[s] / (mask[s]*rowsum_raw[s] + 1e-4*sqrt(len)) computed
    on the host (f64 matvec), so the two [S,S]-sized attention matmuls
    collapse into a Gram matrix:
        weighted = diag(2g)*aspect @ (inp.T @ inp)
    The device computes M = inp.T@inp once (upper-triangular blocks only
    - M is symmetric - with 6 mirror blocks via PE transposes), then
    F2 = 2*ffn_inp + b2 = aTg2-blocks.T @ M + resm2, where
    aTg2 = (2g*aspect).T and resm2 = 2*(inp+aspect)*mask + b2 are host
    inputs.  20+32 matmul-units replace the direct pair's 128.
  - F2 is transposed for the FFN with DMA-engine XBAR transposes (one
    dma_start_transpose per s-block) - zero PE cost.  b2 rides inside
    F2 (b1 is compensated by -0.5*w1@b2); the factor 2 is folded into
    aTg2/resm2 and compensated in w1.
  - FFN1/FFN2 are 32+32 units; phase E normalizes with a per-block
    chain balanced against the 852ns/block PE supply rate: DVE max-evac
    (PSUM), Pool add, ACT square+accum, DVE recip, Pool mul (Pool never
    touches PSUM - a HW restriction).  F2 and the whole normalize chain
    are bf16 (engine ops on bf16 SBUF operands run 2x on DVE).
  - The two final blocks get special tails: the second-to-last drains
    through ACT+Pool; the program-order-last splits its FFN2 into two
    half-column PSUM banks so its fused DVE evac (relu+residual via a
    K=1 ones x b2 PSUM-init matmul) overlaps the last matmul group, and
    its whole chain stays on DVE.  Output is stored bf16 (rows are
    unit-norm; the host upcasts) halving the store DMA.
  - All activations (Copy/Relu/Square/Sqrt) live in one ACT table set,
    loaded once at t=0.  Weight loads ride the Pool SWDGE queue, placed
    in program order so they never delay the ffnT2 DMA-transposes on
    the shared DMA engines; everything else streams on the Sync queue
    in consumption order.

Matmul operands are bf16 (fp32 PSUM accumulation); resm2/F2/fin/out are
bf16 (each ~0.2-0.4% RMS, well inside the 2e-2 gate; measured
end-to-end rel err 3.7e-3 on HW).
"""

import os
import sys

for _p in ("/opt/trn_rl_repo", "/opt/pypackages"):
    if os.path.isdir(_p) and _p not in sys.path:
        sys.path.append(_p)

import numpy as np
import ml_dtypes

BF16 = ml_dtypes.bfloat16

B, S, D = 8, 1024, 512
N_CORES = 8
P = 128                     # SBUF partitions
SB = S // P                 # 8 s-blocks of 128
DB = D // P                 # 4 d-blocks of 128
NF = 512                    # matmul moving free dim (one fp32 PSUM bank)

# --- packed input layouts (element column offsets) -----------------------
# packA (bf16): aTg2 sb-major: [sb][d1][128]  (8 x 512 cols)
A_COLS = SB * D             # 4096
# packB1 (bf16): inpN sb-major: [sb][d]      (8 x 512 cols)
B1_COLS = SB * D            # 4096
# packR (bf16): resm2 sb-major: [sb][d]
R_COLS = SB * D             # 4096
# packI (bf16): identb | negb2 (replicated) | b2 row | ones row
I_IDB = 0
I_NB2 = P                   # 128
I_B2 = I_NB2 + D            # 640  (row 0 only)
I_ONES = I_B2 + D           # 1152 (row 0 only)
I_COLS = I_ONES + P         # 1280
# packB2 (bf16): w1th | w2t
B2_W1 = 0
B2_W2 = DB * D              # 2048
B2_COLS = 2 * DB * D        # 4096
# packF (f32): b1cb
F_COLS = DB                 # 4

_COMPILED = None


def _build():
    import concourse.bacc as bacc
    import concourse.tile as tile
    import concourse.mybir as mybir

    f32 = mybir.dt.float32
    bf16 = mybir.dt.bfloat16
    AF = mybir.ActivationFunctionType
    ALU = mybir.AluOpType

    nc = bacc.Bacc("TRN2", target_bir_lowering=False, debug=False,
                   num_devices=N_CORES)

    packA = nc.dram_tensor("packA", [P, A_COLS], bf16, kind="ExternalInput").ap()
    packB1 = nc.dram_tensor("packB1", [P, B1_COLS], bf16, kind="ExternalInput").ap()
    packR = nc.dram_tensor("packR", [P, R_COLS], bf16, kind="ExternalInput").ap()
    packI = nc.dram_tensor("packI", [P, I_COLS], bf16, kind="ExternalInput").ap()
    packB2 = nc.dram_tensor("packB2", [P, B2_COLS], bf16, kind="ExternalInput").ap()
    packF = nc.dram_tensor("packF", [P, F_COLS], f32, kind="ExternalInput").ap()
    out = nc.dram_tensor("out", [S, D], bf16, kind="ExternalOutput").ap()

    with tile.TileContext(nc) as tc:
        import contextlib
        ctx = contextlib.ExitStack()
        with ctx:
            big = ctx.enter_context(tc.tile_pool(name="big", bufs=1))
            psM = ctx.enter_context(tc.tile_pool(name="psM", bufs=1, space="PSUM"))
            psA = ctx.enter_context(tc.tile_pool(name="psA", bufs=3, space="PSUM"))
            psT = ctx.enter_context(tc.tile_pool(name="psT", bufs=1, space="PSUM"))
            work = ctx.enter_context(tc.tile_pool(name="work", bufs=4))

            # loads in consumption order on the Sync HWDGE queue.
            # w1/w2 are NOT loaded here: their dma_starts are placed on the
            # Pool SWDGE queue further down in program order, so their
            # transfers hit the shared DMA engines only after the early
            # stream has drained and never delay the ffnT2 DMA-transposes.
            gB1 = big.tile([P, B1_COLS], bf16, name="gB1")
            nc.sync.dma_start(gB1[:, 0:D], packB1[:, 0:D])
            nc.sync.dma_start(gB1[:, D:4 * D], packB1[:, D:4 * D])
            nc.sync.dma_start(gB1[:, 4 * D:], packB1[:, 4 * D:])
            gA = big.tile([P, A_COLS], bf16, name="gA")
            nc.sync.dma_start(gA[:, 0:4 * D], packA[:, 0:4 * D])
            gI = big.tile([P, I_COLS], bf16, name="gI")
            nc.sync.dma_start(gI[:], packI[:])
            gF = big.tile([P, F_COLS], f32, name="gF")
            nc.sync.dma_start(gF[:], packF[:])
            gR = big.tile([P, R_COLS], bf16, name="gR")
            nc.sync.dma_start(gR[:, 0:2 * D], packR[:, 0:2 * D])
            nc.sync.dma_start(gA[:, 4 * D:], packA[:, 4 * D:])
            nc.sync.dma_start(gR[:, 2 * D:4 * D], packR[:, 2 * D:4 * D])
            nc.sync.dma_start(gR[:, 4 * D:], packR[:, 4 * D:])
            gB2 = big.tile([P, B2_COLS], bf16, name="gB2")

            # force the single ACT table load (sqrt_and_others:
            # copy/relu/square/sqrt) during the DMA-wait head
            warm = work.tile([P, 1], f32, name="warm_t", tag="warm")
            nc.gpsimd.memset(warm[:], 0.0)
            warm2 = work.tile([P, 1], f32, name="warm2_t", tag="warm2")
            nc.scalar.activation(warm2[:], warm[:], AF.Sqrt)

            # PE warm-up: start the HAM activity window during the DMA head
            wls = work.tile([P, P], bf16, name="wls_t", tag="wls")
            nc.vector.memset(wls[:], 0.0)
            wps = psA.tile([P, NF], f32, name="wps_t", tag="psA")
            for _ in range(4):
                nc.tensor.matmul(wps[:, :P], wls[:], wls[:], start=True,
                                 stop=True)

            def inpN(sb):           # [P, D]
                return gB1[:, sb * D: (sb + 1) * D]

            def aTg(d1, sb):        # [P, P]: lhsT block [d in chunk d1, s-block]
                return gA[:, sb * D + d1 * P: sb * D + (d1 + 1) * P]

            def resm2(sb):          # [P, D] bf16
                return gR[:, sb * D: (sb + 1) * D]

            def w1th(db):           # [P, D]
                return gB2[:, B2_W1 + db * D: B2_W1 + (db + 1) * D]

            def w2t(eb):
                return gB2[:, B2_W2 + eb * D: B2_W2 + (eb + 1) * D]

            identb = gI[:, I_IDB: I_IDB + P]
            negb2 = gI[:, I_NB2: I_NB2 + D]
            b2row = gI[0:1, I_B2: I_B2 + D]
            onesrow = gI[0:1, I_ONES: I_ONES + P]

            def b1col(eb):          # [P, 1] f32
                return gF[:, eb: eb + 1]

            # ---- phase M: M[d1, d2] = sum_s inp[s, d1] * inp[s, d2] ------
            # Upper-triangular blocks only (M is symmetric): stationary
            # block d1 computes columns d2 >= d1.  Mirrors via PE transpose.
            Mps_t = [psM.tile([P, NF], f32, name=f"Mps{d1}",
                              tag=f"psM{d1}") for d1 in range(DB)]
            Mps = [Mps_t[d1][:, 0:NF - d1 * P] for d1 in range(DB)]
            for sb in range(SB):
                for d1 in range(DB):
                    nc.tensor.matmul(
                        Mps[d1],
                        inpN(sb)[:, d1 * P:(d1 + 1) * P],
                        inpN(sb)[:, d1 * P:],
                        start=(sb == 0),
                        stop=(sb == SB - 1),
                    )
            Mb = big.tile([P, DB * NF], bf16, name="Mb")
            # row 0 evacuates in ONE ACT op (single dependency for phase
            # B's first matmul); rows 1-3 split ACT/DVE.  GPSIMD cannot
            # read PSUM on real HW, so only ACT/DVE evacuate.
            nc.scalar.activation(Mb[:, 0:NF], Mps[0], AF.Copy)
            nc.vector.tensor_copy(Mb[:, 1 * NF + 1 * P:2 * NF], Mps[1])
            nc.vector.tensor_copy(Mb[:, 2 * NF + 2 * P:3 * NF], Mps[2])
            nc.scalar.activation(Mb[:, 3 * NF + 3 * P:4 * NF], Mps[3],
                                 AF.Copy)

            # ---- phase B + C interleaved ---------------------------------
            # B: F2 = 2*ffn_inp = aTg2[:,sb].T @ M + resm2, built directly
            #    in bf16 (one DVE add from PSUM; every consumer is bf16)
            # C: ffnT2 = F2.T via one XBAR DMA-transpose per s-block
            F2b_sb = []
            for sb in range(SB):
                F2b_sb.append(big.tile([P, D], bf16, name=f"F2b_sb{sb}"))
            F2p_sb = [big.tile([P, D], bf16, name=f"F2p_sb{i}")
                      for i in range(2)]
            ffnT2 = big.tile([P, DB * S], bf16, name="ffnT2")   # db-major [db*S + s]
            ffnT2_v = ffnT2.rearrange("p (db s) -> p db s", db=DB)
            # 4-bank rotation for the B/D/E matmul groups: 3 from psA
            # plus the psT bank (free once the early mirror is consumed;
            # the first psT use lands well after that)
            _psn = [0]

            def de_psum():
                _psn[0] += 1
                r = _psn[0] % 8
                if r == 0:
                    return psT.tile([P, NF], f32, name="psDE_t", tag="psT")
                if 3 <= r <= 6:
                    return psM.tile([P, NF], f32, name="psDE_t",
                                    tag=f"psM{r - 3}")
                return psA.tile([P, NF], f32, name="psDE_t", tag="psA")

            # Fill the mirror-latency window with real work: the d1=0
            # (pure upper) matmuls of the first two s-blocks run while the
            # mirror transposes + copies complete.
            psB = {}
            for sb in range(2):
                psB[sb] = de_psum()
                nc.tensor.matmul(psB[sb][:], aTg(0, sb),
                                 Mb[:, 0:NF], start=True, stop=False)
                if sb == 0:
                    # mirror the 6 off-diagonal M blocks:
                    # lower(d1,d2) = upper(d2,d1).T
                    mirr = psT.tile([P, 6 * P], bf16, name="mirr_t", tag="psT")
                    k = 0
                    for d1 in range(1, DB):
                        for d2 in range(d1):
                            nc.tensor.transpose(
                                mirr[:, k * P:(k + 1) * P],
                                Mb[:, d2 * NF + d1 * P:d2 * NF + (d1 + 1) * P],
                                identb)
                            k += 1
            # slots (1,0) | (2,0),(2,1) | (3,0),(3,1),(3,2) are contiguous
            # per d1 row -> 3 copies on 3 different engines
            nc.scalar.activation(Mb[:, 1 * NF:1 * NF + P], mirr[:, 0:P],
                                 AF.Copy)
            nc.vector.tensor_copy(Mb[:, 2 * NF:2 * NF + 2 * P],
                                  mirr[:, P:3 * P])
            nc.scalar.activation(Mb[:, 3 * NF:3 * NF + 3 * P],
                                 mirr[:, 3 * P:6 * P], AF.Copy)
            # w1 load rides the Pool queue, time-gated so its transfer
            # never competes with the critical early input stream
            with tc.tile_wait_until(0.0050):
                nc.gpsimd.dma_start(gB2[:, 0:B2_W2], packB2[:, 0:B2_W2])

            for sb in range(SB):
                if sb in psB:
                    ps = psB[sb]
                else:
                    ps = de_psum()
                    nc.tensor.matmul(ps[:], aTg(0, sb),
                                     Mb[:, 0:NF], start=True, stop=False)
                for d1 in range(1, DB):
                    nc.tensor.matmul(
                        ps[:],
                        aTg(d1, sb),
                        Mb[:, d1 * NF:(d1 + 1) * NF],
                        start=False,
                        stop=(d1 == DB - 1),
                    )
                if sb == 3:
                    # w2 load: time-gated; needed only by phase E (~17us)
                    with tc.tile_wait_until(0.0095):
                        nc.gpsimd.dma_start(gB2[:, B2_W2:],
                                            packB2[:, B2_W2:])
                # F2 carries b2 (folded into resm2 on the host; b1 is
                # compensated).  Built directly as bf16: one DVE add from
                # PSUM.  The 2 tail blocks additionally get F2p = F2 - b2
                # for their K=1-init fused fin, far off the critical path.
                nc.vector.tensor_add(F2b_sb[sb][:], ps[:], resm2(sb))
                dstT = ffnT2_v[:, :, sb * P:(sb + 1) * P]
                nc.sync.dma_start_transpose(dstT, F2b_sb[sb][:])
                if sb >= SB - 2:
                    nc.gpsimd.tensor_add(F2p_sb[sb - (SB - 2)][:],
                                         F2b_sb[sb][:], negb2[:])

            # ---- phase D: o1T = relu(w1th.T @ ffnT2 + b1) [e, s] ---------
            o1T_sb = []
            for eb in range(DB):
                o1T_sb.append(big.tile([P, S], bf16, name=f"o1T_sb{eb}"))

            def phaseD(h):
                for eb in range(DB):
                    ps = de_psum()
                    for db in range(DB):
                        nc.tensor.matmul(
                            ps[:],
                            w1th(db)[:, eb * P:(eb + 1) * P],
                            ffnT2[:, db * S + h * NF: db * S + (h + 1) * NF],
                            start=(db == 0),
                            stop=(db == DB - 1),
                        )
                    nc.scalar.activation(
                        o1T_sb[eb][:, h * NF:(h + 1) * NF], ps[:], AF.Relu,
                        bias=b1col(eb), scale=1.0)

            # ---- phase E: o2, final, normalize, store --------------------
            # GPSIMD cannot touch PSUM on HW, so every op reading ps is on
            # DVE or ACT.  Mid-kernel blocks (PE busy) avoid the K=1 b2
            # matmul via max(ps, -b2) + (F2 + b2); tail blocks (PE idle)
            # use the K=1 init and split the whole post-matmul chain into
            # free-dim halves that drain in parallel on DVE vs ACT+Pool
            # (engine op time scales with free size, not partitions).
            def phaseE(sb):
                ps = de_psum()
                tailblk = sb >= SB - 2
                lastblk = sb == SB - 2          # program-order-last block
                h = D // 2
                if lastblk:
                    # column-split PSUM groups in SEPARATE banks: Tile
                    # dependency tracking is per-tile, so cols [0,h) must
                    # live in their own tile for the evac to overlap the
                    # [h,D) matmul group
                    psB = de_psum()
                    for pst, c0, c1 in ((ps, 0, h), (psB, h, D)):
                        nc.tensor.matmul(pst[:, 0:h], onesrow,
                                         b2row[:, c0:c1],
                                         start=True, stop=False)
                        for eb in range(DB):
                            nc.tensor.matmul(
                                pst[:, 0:h],
                                o1T_sb[eb][:, sb * P:(sb + 1) * P],
                                w2t(eb)[:, c0:c1],
                                start=False,
                                stop=(eb == DB - 1),
                            )
                else:
                    if tailblk:
                        nc.tensor.matmul(ps[:], onesrow, b2row,
                                         start=True, stop=False)
                    for eb in range(DB):
                        nc.tensor.matmul(
                            ps[:],
                            o1T_sb[eb][:, sb * P:(sb + 1) * P],
                            w2t(eb),
                            start=(not tailblk and eb == 0),
                            stop=(eb == DB - 1),
                        )
                tt = work.tile([P, D], bf16, name="tt_t", tag="tt")
                fin = work.tile([P, D], bf16, name="fin_t", tag="fin")
                sq = work.tile([P, D], bf16, name="sq_t", tag="sq")
                ss = work.tile([P, 4], f32, name="ss_t", tag="ss")
                rn = work.tile([P, 1], f32, name="rn_t", tag="rn")
                rr = work.tile([P, 1], f32, name="rr_t", tag="rr")
                ot = work.tile([P, D], bf16, name="ot_t", tag="ot")
                if not tailblk:
                    # balanced chain: per block DVE ~718, ACT ~672,
                    # Pool ~854 vs the 852ns PE supply rate of phase E
                    # fin = max(ps, -b2) + F2 (F2 carries b2):
                    # DVE max (psum) -> Pool add (sbuf)
                    nc.vector.tensor_max(tt[:], ps[:], negb2[:])
                    nc.gpsimd.tensor_add(fin[:], tt[:], F2b_sb[sb][:])
                    # ACT square+sqrt -> DVE recip -> Pool mul
                    nc.scalar.activation(sq[:], fin[:], AF.Square,
                                         accum_out=ss[:, 0:1])
                    nc.scalar.activation(rn[:], ss[:, 0:1], AF.Sqrt)
                    nc.vector.reciprocal(rr[:], rn[:])
                    nc.gpsimd.tensor_scalar_mul(ot[:], fin[:], rr[:])
                    # mid stores ride the Sync queue (SP is idle)
                    nc.sync.dma_start(out[sb * P:(sb + 1) * P, :], ot[:])
                    return
                # tail blocks: fin = relu(ps) + (F2 - b2)
                F2p = F2p_sb[sb - (SB - 2)]
                if not lastblk:
                    # second-to-last block: ACT+Pool chain so DVE stays
                    # free for the program-order-last block's chain
                    nc.scalar.activation(tt[:], ps[:], AF.Relu)
                    nc.gpsimd.tensor_add(fin[:], tt[:], F2p[:])
                    nc.scalar.activation(sq[:], fin[:], AF.Square,
                                         accum_out=ss[:, 0:1])
                    nc.scalar.activation(rn[:], ss[:, 0:1], AF.Sqrt)
                    nc.vector.reciprocal(rr[:], rn[:])
                    # mul on ACT (Copy*scale): Pool is still draining the
                    # mid blocks' adds/muls at this point
                    nc.scalar.activation(ot[:], fin[:], AF.Copy,
                                         bias=0.0, scale=rr[:])
                    nc.scalar.dma_start(out[sb * P:(sb + 1) * P, :], ot[:])
                    return
                # last block: DVE-only chain, column halves pipelined
                # against the split matmul groups (evac of cols [0,h)
                # runs while cols [h,D) are still in the PE)
                nc.vector.scalar_tensor_tensor(
                    fin[:, :h], ps[:, 0:h], 0.0, F2p[:, :h],
                    ALU.max, ALU.add)
                nc.vector.scalar_tensor_tensor(
                    sq[:, :h], fin[:, :h], 1.0, fin[:, :h],
                    ALU.mult, ALU.mult, accum_out=ss[:, 0:1])
                nc.vector.scalar_tensor_tensor(
                    fin[:, h:], psB[:, 0:h], 0.0, F2p[:, h:],
                    ALU.max, ALU.add)
                nc.vector.scalar_tensor_tensor(
                    sq[:, h:], fin[:, h:], 1.0, fin[:, h:],
                    ALU.mult, ALU.mult, accum_out=ss[:, 1:2])
                nc.vector.tensor_add(ss[:, 2:3], ss[:, 0:1], ss[:, 1:2])
                nc.scalar.activation(rn[:], ss[:, 2:3], AF.Sqrt)
                nc.vector.reciprocal(rr[:], rn[:])
                nc.vector.tensor_scalar_mul(ot[:], fin[:], rr[:])
                nc.sync.dma_start(out[sb * P:(sb + 1) * P, :], ot[:])

            phaseD(0)
            phaseE(0)
            phaseE(1)
            phaseD(1)
            for sb in (2, 3, 4, 5, 7, 6):
                phaseE(sb)

    nc.compile()
    return nc


def _get_compiled():
    global _COMPILED
    if _COMPILED is None:
        _COMPILED = _build()
    return _COMPILED


def _host_prep(inp, inp_len, aspect, w1, b1, w2, b2):
    inp = np.asarray(inp, dtype=np.float32)
    aspect = np.asarray(aspect, dtype=np.float32)
    inp_len = np.asarray(inp_len, dtype=np.float32)
    w1 = np.asarray(w1, dtype=np.float32)
    b1 = np.asarray(b1, dtype=np.float32)
    w2 = np.asarray(w2, dtype=np.float32)
    b2 = np.asarray(b2, dtype=np.float32)

    packB2 = np.zeros((P, B2_COLS), dtype=BF16)
    w1th = (w1.T * 0.5).astype(BF16)                 # [d, e]
    w2t = w2.T.astype(BF16)                          # [e, f]
    for db in range(DB):
        packB2[:, B2_W1 + db * D: B2_W1 + (db + 1) * D] = \
            w1th[db * P:(db + 1) * P, :]
        packB2[:, B2_W2 + db * D: B2_W2 + (db + 1) * D] = \
            w2t[db * P:(db + 1) * P, :]
    packI = np.zeros((P, I_COLS), dtype=BF16)
    packI[:, I_IDB: I_IDB + P] = np.eye(P).astype(BF16)
    packI[:, I_NB2: I_NB2 + D] = np.tile((-b2).astype(BF16), (P, 1))
    packI[0, I_B2: I_B2 + D] = b2.astype(BF16)
    packI[0, I_ONES: I_ONES + P] = np.ones(P, dtype=BF16)

    # relu(x + b2) = max(x, -b2) + b2: b2 is folded into resm2 (so F2
    # carries it), and b1 compensates for the extra b2/2 entering FFN1
    b1c = b1.astype(np.float64) - 0.5 * (w1.astype(np.float64) @
                                         b2.astype(np.float64))
    b1cb = b1c.reshape(DB, P).T.astype(np.float32)   # [P, DB]

    in_maps = []
    for bidx in range(B):
        x = inp[bidx].astype(np.float64)             # [S, D]
        a = aspect[bidx].astype(np.float64)
        ln = float(inp_len[bidx])
        scale = np.sqrt(ln)
        mask = (np.arange(S) < int(ln)).astype(np.float64)
        rowsum = a @ x.sum(axis=0)
        g = mask / (mask * rowsum + 1e-4 * scale)
        aTg2 = ((a * (2.0 * g)[:, None]).T).astype(BF16)   # [D, S]
        resm2 = (2.0 * (x + a) * mask[:, None]
                 + b2.astype(np.float64)[None, :])         # [S, D] (+b2)

        pA = np.empty((P, A_COLS), dtype=BF16)
        for sb in range(SB):
            for d1 in range(DB):
                pA[:, sb * D + d1 * P: sb * D + (d1 + 1) * P] = \
                    aTg2[d1 * P:(d1 + 1) * P, sb * P:(sb + 1) * P]

        pB1 = np.empty((P, B1_COLS), dtype=BF16)
        xb = x.astype(BF16)
        for sb in range(SB):
            pB1[:, sb * D:(sb + 1) * D] = xb[sb * P:(sb + 1) * P, :]

        pR = np.empty((P, R_COLS), dtype=BF16)
        rb = resm2.astype(BF16)
        for sb in range(SB):
            pR[:, sb * D:(sb + 1) * D] = rb[sb * P:(sb + 1) * P, :]

        pF = np.zeros((P, F_COLS), dtype=np.float32)
        pF[:, 0:DB] = b1cb

        in_maps.append({"packA": pA, "packB1": pB1, "packR": pR,
                        "packI": packI, "packB2": packB2, "packF": pF})
    return in_maps


def kernel(inp, inp_len, aspect, w1, b1, w2, b2):
    from concourse.bass_utils import run_bass_kernel_spmd

    nc = _get_compiled()
    in_maps = _host_prep(inp, inp_len, aspect, w1, b1, w2, b2)
    res = run_bass_kernel_spmd(nc, in_maps, core_ids=list(range(N_CORES)))
    # device stores bf16; rows are unit-norm so bf16 rounding adds ~0.2%
    return np.stack([res.results[i]["out"] for i in range(N_CORES)],
                    axis=0).astype(np.float32)



# revision 25
# speedup vs baseline: 1.4375x; 1.4375x over previous
"""Trainium2 Bass kernel for the AttentiveModule problem.

Reference computation (per batch element b, S=1024, D=512):
    att   = aspect @ inp.T / sqrt(len)                # [S,S]
    exp   = att * mask[:, None]                       # row mask (query dim)
    att_n = exp / (exp.sum(-1, keepdims=True) + 1e-4) # linear normalize
    w     = att_n @ inp                               # [S,D]
    ffn_inp = w + (inp + aspect) * mask[:, None]
    o1    = relu(ffn_inp @ w1.T + b1)
    o2    = relu(o1 @ w2.T + b2)
    final = 2*ffn_inp + o2
    out   = final / ||final||_2(axis=-1)

Sharding: data-parallel over batch, one batch element per NeuronCore
(8 cores).  Host prep is O(B*S*D), trivial vs the O(S*D^2) device work.

Algorithm (per core):
  - The row mask and linear normalization factor into a per-row scalar
    g[s] = mask[s] / (mask[s]*rowsum_raw[s] + 1e-4*sqrt(len)) computed
    on the host (f64 matvec), so the two [S,S]-sized attention matmuls
    collapse into a Gram matrix:
        weighted = diag(2g)*aspect @ (inp.T @ inp)
    The device computes M = inp.T@inp once (upper-triangular blocks only
    - M is symmetric - with 6 mirror blocks via PE transposes), then
    F2 = 2*ffn_inp + b2 = aTg2-blocks.T @ M + resm2, where
    aTg2 = (2g*aspect).T and resm2 = 2*(inp+aspect)*mask + b2 are host
    inputs.  20+32 matmul-units replace the direct pair's 128.
  - F2 is transposed for the FFN with DMA-engine XBAR transposes (one
    dma_start_transpose per s-block) - zero PE cost.  b2 rides inside
    F2 (b1 is compensated by -0.5*w1@b2); the factor 2 is folded into
    aTg2/resm2 and compensated in w1.
  - FFN1/FFN2 are 32+32 units; phase E normalizes with a per-block
    chain balanced against the 852ns/block PE supply rate: DVE max-evac
    (PSUM), Pool add, ACT square+accum, DVE recip, Pool mul (Pool never
    touches PSUM - a HW restriction).  F2 and the whole normalize chain
    are bf16 (engine ops on bf16 SBUF operands run 2x on DVE).
  - The two final blocks get special tails: the second-to-last drains
    through ACT+Pool; the program-order-last splits its FFN2 into two
    half-column PSUM banks so its fused DVE evac (relu+residual via a
    K=1 ones x b2 PSUM-init matmul) overlaps the last matmul group, and
    its whole chain stays on DVE.  Output is stored bf16 (rows are
    unit-norm; the host upcasts) halving the store DMA.
  - All activations (Copy/Relu/Square/Sqrt) live in one ACT table set,
    loaded once at t=0.  Weight loads ride the Pool SWDGE queue, placed
    in program order so they never delay the ffnT2 DMA-transposes on
    the shared DMA engines; everything else streams on the Sync queue
    in consumption order.

Matmul operands are bf16 (fp32 PSUM accumulation); resm2/F2/fin/out are
bf16 (each ~0.2-0.4% RMS, well inside the 2e-2 gate; measured
end-to-end rel err 3.7e-3 on HW).
"""

import os
import sys

for _p in ("/opt/trn_rl_repo", "/opt/pypackages"):
    if os.path.isdir(_p) and _p not in sys.path:
        sys.path.append(_p)

import numpy as np
import ml_dtypes

BF16 = ml_dtypes.bfloat16

B, S, D = 8, 1024, 512
N_CORES = 8
P = 128                     # SBUF partitions
SB = S // P                 # 8 s-blocks of 128
DB = D // P                 # 4 d-blocks of 128
NF = 512                    # matmul moving free dim (one fp32 PSUM bank)

# --- packed input layouts (element column offsets) -----------------------
# packA (bf16): a2 = 2g*aspect, PLAIN [s,d] layout, sb-major (the device
# transposes it with XBAR DMA transposes - saves shipping a separate
# transposed copy and lets resm2 be derived on device)
A_COLS = SB * D             # 4096
# packB1 (bf16): inpN sb-major: [sb][d]      (8 x 512 cols)
B1_COLS = SB * D            # 4096
# packI (bf16): identb | negb2 (replicated) | b2 row | ones row
I_IDB = 0
I_NB2 = P                   # 128
I_B2 = I_NB2 + D            # 640  (row 0 only)
I_ONES = I_B2 + D           # 1152 (row 0 only)
I_COLS = I_ONES + P         # 1280
# packB2 (bf16): w1th | w2t
B2_W1 = 0
B2_W2 = DB * D              # 2048
B2_COLS = 2 * DB * D        # 4096
# packF (f32): b1cb | h2col (mask/(2g), per-partition per sb) | m2col
F_B1C = 0
F_H2 = DB                   # 4
F_M2 = DB + SB              # 12
F_COLS = DB + 2 * SB        # 20

_COMPILED = None


def _build():
    import concourse.bacc as bacc
    import concourse.tile as tile
    import concourse.mybir as mybir

    f32 = mybir.dt.float32
    bf16 = mybir.dt.bfloat16
    AF = mybir.ActivationFunctionType
    ALU = mybir.AluOpType

    nc = bacc.Bacc("TRN2", target_bir_lowering=False, debug=False,
                   num_devices=N_CORES)

    packA = nc.dram_tensor("packA", [P, A_COLS], bf16, kind="ExternalInput").ap()
    packB1 = nc.dram_tensor("packB1", [P, B1_COLS], bf16, kind="ExternalInput").ap()
    packI = nc.dram_tensor("packI", [P, I_COLS], bf16, kind="ExternalInput").ap()
    packB2 = nc.dram_tensor("packB2", [P, B2_COLS], bf16, kind="ExternalInput").ap()
    packF = nc.dram_tensor("packF", [P, F_COLS], f32, kind="ExternalInput").ap()
    out = nc.dram_tensor("out", [S, D], bf16, kind="ExternalOutput").ap()

    with tile.TileContext(nc) as tc:
        import contextlib
        ctx = contextlib.ExitStack()
        with ctx:
            big = ctx.enter_context(tc.tile_pool(name="big", bufs=1))
            psM = ctx.enter_context(tc.tile_pool(name="psM", bufs=1, space="PSUM"))
            psA = ctx.enter_context(tc.tile_pool(name="psA", bufs=3, space="PSUM"))
            psT = ctx.enter_context(tc.tile_pool(name="psT", bufs=1, space="PSUM"))
            work = ctx.enter_context(tc.tile_pool(name="work", bufs=4))

            # loads in consumption order on the Sync HWDGE queue.
            # w1/w2 are NOT loaded here: their dma_starts are placed on the
            # Pool SWDGE queue further down in program order, so their
            # transfers hit the shared DMA engines only after the early
            # stream has drained and never delay the ffnT2 DMA-transposes.
            gB1 = big.tile([P, B1_COLS], bf16, name="gB1")
            nc.sync.dma_start(gB1[:, 0:D], packB1[:, 0:D])
            nc.sync.dma_start(gB1[:, D:4 * D], packB1[:, D:4 * D])
            nc.sync.dma_start(gB1[:, 4 * D:], packB1[:, 4 * D:])
            gI = big.tile([P, I_COLS], bf16, name="gI")
            nc.sync.dma_start(gI[:], packI[:])
            gF = big.tile([P, F_COLS], f32, name="gF")
            nc.sync.dma_start(gF[:], packF[:])
            # a2 loads per-sb, each immediately followed by its XBAR
            # transpose into gAT (stationary layout for phase B); phase B
            # sb_i gates on transpose i, so interleaving keeps the first
            # transposes as early as possible on the Sync queue.
            gA = big.tile([P, A_COLS], bf16, name="gA")
            gAT = big.tile([P, DB * S], bf16, name="gAT")
            gAT_v = gAT.rearrange("p (db s) -> p db s", db=DB)
            for sb in range(SB):
                nc.sync.dma_start(gA[:, sb * D:(sb + 1) * D],
                                  packA[:, sb * D:(sb + 1) * D])
                nc.sync.dma_start_transpose(
                    gAT_v[:, :, sb * P:(sb + 1) * P],
                    gA[:, sb * D:(sb + 1) * D])
            gB2 = big.tile([P, B2_COLS], bf16, name="gB2")
            gRm = big.tile([P, SB * D], bf16, name="gRm")

            # force the single ACT table load (sqrt_and_others:
            # copy/relu/square/sqrt) during the DMA-wait head
            warm = work.tile([P, 1], f32, name="warm_t", tag="warm")
            nc.gpsimd.memset(warm[:], 0.0)
            warm2 = work.tile([P, 1], f32, name="warm2_t", tag="warm2")
            nc.scalar.activation(warm2[:], warm[:], AF.Sqrt)

            # PE warm-up: start the HAM activity window during the DMA head
            wls = work.tile([P, P], bf16, name="wls_t", tag="wls")
            nc.vector.memset(wls[:], 0.0)
            wps = psA.tile([P, NF], f32, name="wps_t", tag="psA")
            for _ in range(4):
                nc.tensor.matmul(wps[:, :P], wls[:], wls[:], start=True,
                                 stop=True)

            def inpN(sb):           # [P, D]
                return gB1[:, sb * D: (sb + 1) * D]

            def aTg(d1, sb):        # [P, P]: lhsT block [d in chunk d1, s-block]
                return gAT[:, d1 * S + sb * P: d1 * S + (sb + 1) * P]

            def resm2(sb):          # [P, D] bf16 (device-derived)
                return gRm[:, sb * D: (sb + 1) * D]

            def w1th(db):           # [P, D]
                return gB2[:, B2_W1 + db * D: B2_W1 + (db + 1) * D]

            def w2t(eb):
                return gB2[:, B2_W2 + eb * D: B2_W2 + (eb + 1) * D]

            identb = gI[:, I_IDB: I_IDB + P]
            negb2 = gI[:, I_NB2: I_NB2 + D]
            b2row = gI[0:1, I_B2: I_B2 + D]
            onesrow = gI[0:1, I_ONES: I_ONES + P]

            def b1col(eb):          # [P, 1] f32
                return gF[:, F_B1C + eb: F_B1C + eb + 1]

            def h2col(sb):          # [P, 1] f32: mask/(2g) for s-block sb
                return gF[:, F_H2 + sb: F_H2 + sb + 1]

            def m2col(sb):          # [P, 1] f32: 2*mask for s-block sb
                return gF[:, F_M2 + sb: F_M2 + sb + 1]

            def prep_resm2(sb):
                # resm2 = 2*(x+a)*mask + b2, derived from a2 = 2g*a:
                #   t = a2*h2 + x   (h2 = mask/(2g); masked rows -> t = x)
                #   resm2 = t*m2 - (-b2)
                # Pool does the two tensor ops, DVE the fused stt finish.
                tx = work.tile([P, D], bf16, name="tx_t", tag="tx")
                ty = work.tile([P, D], bf16, name="ty_t", tag="ty")
                nc.gpsimd.tensor_scalar_mul(tx[:], gA[:, sb * D:(sb + 1) * D],
                                            h2col(sb))
                nc.gpsimd.tensor_add(ty[:], tx[:], inpN(sb))
                return ty

            def finish_resm2(sb, ty):
                nc.vector.scalar_tensor_tensor(
                    gRm[:, sb * D:(sb + 1) * D], ty[:], m2col(sb), negb2[:],
                    ALU.mult, ALU.subtract)

            # ---- phase M: M[d1, d2] = sum_s inp[s, d1] * inp[s, d2] ------
            # Upper-triangular blocks only (M is symmetric): stationary
            # block d1 computes columns d2 >= d1.  Mirrors via PE transpose.
            Mps_t = [psM.tile([P, NF], f32, name=f"Mps{d1}",
                              tag=f"psM{d1}") for d1 in range(DB)]
            Mps = [Mps_t[d1][:, 0:NF - d1 * P] for d1 in range(DB)]
            for sb in range(SB):
                for d1 in range(DB):
                    nc.tensor.matmul(
                        Mps[d1],
                        inpN(sb)[:, d1 * P:(d1 + 1) * P],
                        inpN(sb)[:, d1 * P:],
                        start=(sb == 0),
                        stop=(sb == SB - 1),
                    )
            Mb = big.tile([P, DB * NF], bf16, name="Mb")
            # row 0 evacuates in ONE ACT op (single dependency for phase
            # B's first matmul); rows 1-3 split ACT/DVE.  GPSIMD cannot
            # read PSUM on real HW, so only ACT/DVE evacuate.
            nc.scalar.activation(Mb[:, 0:NF], Mps[0], AF.Copy)
            nc.vector.tensor_copy(Mb[:, 1 * NF + 1 * P:2 * NF], Mps[1])
            nc.vector.tensor_copy(Mb[:, 2 * NF + 2 * P:3 * NF], Mps[2])
            nc.scalar.activation(Mb[:, 3 * NF + 3 * P:4 * NF], Mps[3],
                                 AF.Copy)

            # ---- phase B + C interleaved ---------------------------------
            # B: F2 = 2*ffn_inp = aTg2[:,sb].T @ M + resm2, built directly
            #    in bf16 (one DVE add from PSUM; every consumer is bf16)
            # C: ffnT2 = F2.T via one XBAR DMA-transpose per s-block
            F2b_sb = []
            for sb in range(SB):
                F2b_sb.append(big.tile([P, D], bf16, name=f"F2b_sb{sb}"))
            F2p_sb = [big.tile([P, D], bf16, name=f"F2p_sb{i}")
                      for i in range(2)]
            ffnT2 = big.tile([P, DB * S], bf16, name="ffnT2")   # db-major [db*S + s]
            ffnT2_v = ffnT2.rearrange("p (db s) -> p db s", db=DB)
            # 4-bank rotation for the B/D/E matmul groups: 3 from psA
            # plus the psT bank (free once the early mirror is consumed;
            # the first psT use lands well after that)
            _psn = [0]

            def de_psum():
                _psn[0] += 1
                r = _psn[0] % 8
                if r == 0:
                    return psT.tile([P, NF], f32, name="psDE_t", tag="psT")
                if 3 <= r <= 6:
                    return psM.tile([P, NF], f32, name="psDE_t",
                                    tag=f"psM{r - 3}")
                return psA.tile([P, NF], f32, name="psDE_t", tag="psA")

            # Fill the mirror-latency window with real work: the d1=0
            # (pure upper) matmuls of the first two s-blocks run while the
            # mirror transposes + copies complete.
            psB = {}
            for sb in range(2):
                psB[sb] = de_psum()
                nc.tensor.matmul(psB[sb][:], aTg(0, sb),
                                 Mb[:, 0:NF], start=True, stop=False)
                if sb == 0:
                    # mirror the 6 off-diagonal M blocks:
                    # lower(d1,d2) = upper(d2,d1).T
                    mirr = psT.tile([P, 6 * P], bf16, name="mirr_t", tag="psT")
                    k = 0
                    for d1 in range(1, DB):
                        for d2 in range(d1):
                            nc.tensor.transpose(
                                mirr[:, k * P:(k + 1) * P],
                                Mb[:, d2 * NF + d1 * P:d2 * NF + (d1 + 1) * P],
                                identb)
                            k += 1
            # slots (1,0) | (2,0),(2,1) | (3,0),(3,1),(3,2) are contiguous
            # per d1 row -> 3 copies on 3 different engines
            nc.scalar.activation(Mb[:, 1 * NF:1 * NF + P], mirr[:, 0:P],
                                 AF.Copy)
            nc.vector.tensor_copy(Mb[:, 2 * NF:2 * NF + 2 * P],
                                  mirr[:, P:3 * P])
            nc.scalar.activation(Mb[:, 3 * NF:3 * NF + 3 * P],
                                 mirr[:, 3 * P:6 * P], AF.Copy)
            # w1 load rides the Pool queue, time-gated so its transfer
            # never competes with the critical early input stream
            with tc.tile_wait_until(0.0050):
                nc.gpsimd.dma_start(gB2[:, 0:B2_W2], packB2[:, 0:B2_W2])

            # resm2 for the first two s-blocks before phase B consumes it;
            # later blocks are prepped two iterations ahead inside the loop
            for psb in (0, 1):
                finish_resm2(psb, prep_resm2(psb))

            for sb in range(SB):
                if sb in psB:
                    ps = psB[sb]
                else:
                    ps = de_psum()
                    nc.tensor.matmul(ps[:], aTg(0, sb),
                                     Mb[:, 0:NF], start=True, stop=False)
                for d1 in range(1, DB):
                    nc.tensor.matmul(
                        ps[:],
                        aTg(d1, sb),
                        Mb[:, d1 * NF:(d1 + 1) * NF],
                        start=False,
                        stop=(d1 == DB - 1),
                    )
                if sb == 3:
                    # w2 load: time-gated; needed only by phase E (~17us)
                    with tc.tile_wait_until(0.0095):
                        nc.gpsimd.dma_start(gB2[:, B2_W2:],
                                            packB2[:, B2_W2:])
                # F2 carries b2 (folded into resm2; b1 is compensated).
                # Built directly as bf16: one DVE add from PSUM.  The 2
                # tail blocks additionally get F2p = F2 - b2 for their
                # K=1-init fused fin, far off the critical path.
                nc.vector.tensor_add(F2b_sb[sb][:], ps[:], resm2(sb))
                # ffnT2 transposes ride the ACT HWDGE queue: the Sync
                # queue is occupied by the a2 loads + aT transposes
                dstT = ffnT2_v[:, :, sb * P:(sb + 1) * P]
                nc.scalar.dma_start_transpose(dstT, F2b_sb[sb][:])
                if sb >= SB - 2:
                    nc.gpsimd.tensor_add(F2p_sb[sb - (SB - 2)][:],
                                         F2b_sb[sb][:], negb2[:])
                if sb + 2 < SB:
                    finish_resm2(sb + 2, prep_resm2(sb + 2))

            # ---- phase D: o1T = relu(w1th.T @ ffnT2 + b1) [e, s] ---------
            o1T_sb = []
            for eb in range(DB):
                o1T_sb.append(big.tile([P, S], bf16, name=f"o1T_sb{eb}"))

            def phaseD(h):
                for eb in range(DB):
                    ps = de_psum()
                    for db in range(DB):
                        nc.tensor.matmul(
                            ps[:],
                            w1th(db)[:, eb * P:(eb + 1) * P],
                            ffnT2[:, db * S + h * NF: db * S + (h + 1) * NF],
                            start=(db == 0),
                            stop=(db == DB - 1),
                        )
                    nc.scalar.activation(
                        o1T_sb[eb][:, h * NF:(h + 1) * NF], ps[:], AF.Relu,
                        bias=b1col(eb), scale=1.0)

            # ---- phase E: o2, final, normalize, store --------------------
            # GPSIMD cannot touch PSUM on HW, so every op reading ps is on
            # DVE or ACT.  Mid-kernel blocks (PE busy) avoid the K=1 b2
            # matmul via max(ps, -b2) + (F2 + b2); tail blocks (PE idle)
            # use the K=1 init and split the whole post-matmul chain into
            # free-dim halves that drain in parallel on DVE vs ACT+Pool
            # (engine op time scales with free size, not partitions).
            def phaseE(sb):
                ps = de_psum()
                tailblk = sb >= SB - 2
                lastblk = sb == SB - 2          # program-order-last block
                h = D // 2
                if lastblk:
                    # column-split PSUM groups in SEPARATE banks: Tile
                    # dependency tracking is per-tile, so cols [0,h) must
                    # live in their own tile for the evac to overlap the
                    # [h,D) matmul group
                    psB = de_psum()
                    for pst, c0, c1 in ((ps, 0, h), (psB, h, D)):
                        nc.tensor.matmul(pst[:, 0:h], onesrow,
                                         b2row[:, c0:c1],
                                         start=True, stop=False)
                        for eb in range(DB):
                            nc.tensor.matmul(
                                pst[:, 0:h],
                                o1T_sb[eb][:, sb * P:(sb + 1) * P],
                                w2t(eb)[:, c0:c1],
                                start=False,
                                stop=(eb == DB - 1),
                            )
                else:
                    if tailblk:
                        nc.tensor.matmul(ps[:], onesrow, b2row,
                                         start=True, stop=False)
                    for eb in range(DB):
                        nc.tensor.matmul(
                            ps[:],
                            o1T_sb[eb][:, sb * P:(sb + 1) * P],
                            w2t(eb),
                            start=(not tailblk and eb == 0),
                            stop=(eb == DB - 1),
                        )
                tt = work.tile([P, D], bf16, name="tt_t", tag="tt")
                fin = work.tile([P, D], bf16, name="fin_t", tag="fin")
                sq = work.tile([P, D], bf16, name="sq_t", tag="sq")
                ss = work.tile([P, 4], f32, name="ss_t", tag="ss")
                rn = work.tile([P, 1], f32, name="rn_t", tag="rn")
                rr = work.tile([P, 1], f32, name="rr_t", tag="rr")
                ot = work.tile([P, D], bf16, name="ot_t", tag="ot")
                if not tailblk:
                    # balanced chain: per block DVE ~718, ACT ~672,
                    # Pool ~854 vs the 852ns PE supply rate of phase E
                    # fin = max(ps, -b2) + F2 (F2 carries b2):
                    # DVE max (psum) -> Pool add (sbuf)
                    nc.vector.tensor_max(tt[:], ps[:], negb2[:])
                    nc.gpsimd.tensor_add(fin[:], tt[:], F2b_sb[sb][:])
                    # ACT square+sqrt -> DVE recip -> Pool mul
                    nc.scalar.activation(sq[:], fin[:], AF.Square,
                                         accum_out=ss[:, 0:1])
                    nc.scalar.activation(rn[:], ss[:, 0:1], AF.Sqrt)
                    nc.vector.reciprocal(rr[:], rn[:])
                    nc.gpsimd.tensor_scalar_mul(ot[:], fin[:], rr[:])
                    # mid stores ride the Sync queue (SP is idle)
                    nc.sync.dma_start(out[sb * P:(sb + 1) * P, :], ot[:])
                    return
                # tail blocks: fin = relu(ps) + (F2 - b2)
                F2p = F2p_sb[sb - (SB - 2)]
                if not lastblk:
                    # second-to-last block: ACT+Pool chain so DVE stays
                    # free for the program-order-last block's chain
                    nc.scalar.activation(tt[:], ps[:], AF.Relu)
                    nc.gpsimd.tensor_add(fin[:], tt[:], F2p[:])
                    nc.scalar.activation(sq[:], fin[:], AF.Square,
                                         accum_out=ss[:, 0:1])
                    nc.scalar.activation(rn[:], ss[:, 0:1], AF.Sqrt)
                    nc.vector.reciprocal(rr[:], rn[:])
                    # mul on ACT (Copy*scale): Pool is still draining the
                    # mid blocks' adds/muls at this point
                    nc.scalar.activation(ot[:], fin[:], AF.Copy,
                                         bias=0.0, scale=rr[:])
                    nc.scalar.dma_start(out[sb * P:(sb + 1) * P, :], ot[:])
                    return
                # last block: DVE-only chain, column halves pipelined
                # against the split matmul groups (evac of cols [0,h)
                # runs while cols [h,D) are still in the PE)
                nc.vector.scalar_tensor_tensor(
                    fin[:, :h], ps[:, 0:h], 0.0, F2p[:, :h],
                    ALU.max, ALU.add)
                nc.vector.scalar_tensor_tensor(
                    sq[:, :h], fin[:, :h], 1.0, fin[:, :h],
                    ALU.mult, ALU.mult, accum_out=ss[:, 0:1])
                nc.vector.scalar_tensor_tensor(
                    fin[:, h:], psB[:, 0:h], 0.0, F2p[:, h:],
                    ALU.max, ALU.add)
                nc.vector.scalar_tensor_tensor(
                    sq[:, h:], fin[:, h:], 1.0, fin[:, h:],
                    ALU.mult, ALU.mult, accum_out=ss[:, 1:2])
                nc.vector.tensor_add(ss[:, 2:3], ss[:, 0:1], ss[:, 1:2])
                nc.scalar.activation(rn[:], ss[:, 2:3], AF.Sqrt)
                nc.vector.reciprocal(rr[:], rn[:])
                nc.vector.tensor_scalar_mul(ot[:], fin[:], rr[:])
                nc.sync.dma_start(out[sb * P:(sb + 1) * P, :], ot[:])

            phaseD(0)
            phaseE(0)
            phaseE(1)
            phaseD(1)
            for sb in (2, 3, 4, 5, 7, 6):
                phaseE(sb)

    nc.compile()
    return nc


def _get_compiled():
    global _COMPILED
    if _COMPILED is None:
        _COMPILED = _build()
    return _COMPILED


def _host_prep(inp, inp_len, aspect, w1, b1, w2, b2):
    inp = np.asarray(inp, dtype=np.float32)
    aspect = np.asarray(aspect, dtype=np.float32)
    inp_len = np.asarray(inp_len, dtype=np.float32)
    w1 = np.asarray(w1, dtype=np.float32)
    b1 = np.asarray(b1, dtype=np.float32)
    w2 = np.asarray(w2, dtype=np.float32)
    b2 = np.asarray(b2, dtype=np.float32)

    packB2 = np.zeros((P, B2_COLS), dtype=BF16)
    w1th = (w1.T * 0.5).astype(BF16)                 # [d, e]
    w2t = w2.T.astype(BF16)                          # [e, f]
    for db in range(DB):
        packB2[:, B2_W1 + db * D: B2_W1 + (db + 1) * D] = \
            w1th[db * P:(db + 1) * P, :]
        packB2[:, B2_W2 + db * D: B2_W2 + (db + 1) * D] = \
            w2t[db * P:(db + 1) * P, :]
    packI = np.zeros((P, I_COLS), dtype=BF16)
    packI[:, I_IDB: I_IDB + P] = np.eye(P).astype(BF16)
    packI[:, I_NB2: I_NB2 + D] = np.tile((-b2).astype(BF16), (P, 1))
    packI[0, I_B2: I_B2 + D] = b2.astype(BF16)
    packI[0, I_ONES: I_ONES + P] = np.ones(P, dtype=BF16)

    # relu(x + b2) = max(x, -b2) + b2: b2 is folded into resm2 (so F2
    # carries it), and b1 compensates for the extra b2/2 entering FFN1
    b1c = b1.astype(np.float64) - 0.5 * (w1.astype(np.float64) @
                                         b2.astype(np.float64))
    b1cb = b1c.reshape(DB, P).T.astype(np.float32)   # [P, DB]

    in_maps = []
    for bidx in range(B):
        x = inp[bidx].astype(np.float64)             # [S, D]
        a = aspect[bidx].astype(np.float64)
        ln = float(inp_len[bidx])
        scale = np.sqrt(ln)
        mask = (np.arange(S) < int(ln)).astype(np.float64)
        rowsum = a @ x.sum(axis=0)
        g = mask / (mask * rowsum + 1e-4 * scale)
        a2 = a * (2.0 * g)[:, None]                  # [S, D]
        # h2 = mask/(2g): recovers a from a2 on device (0 on masked rows,
        # where resm2 = b2 regardless)
        with np.errstate(divide="ignore", invalid="ignore"):
            h2 = np.where(mask > 0, 1.0 / (2.0 * g), 0.0)

        pA = np.empty((P, A_COLS), dtype=BF16)
        a2b = a2.astype(BF16)
        for sb in range(SB):
            pA[:, sb * D:(sb + 1) * D] = a2b[sb * P:(sb + 1) * P, :]

        pB1 = np.empty((P, B1_COLS), dtype=BF16)
        xb = x.astype(BF16)
        for sb in range(SB):
            pB1[:, sb * D:(sb + 1) * D] = xb[sb * P:(sb + 1) * P, :]

        pF = np.zeros((P, F_COLS), dtype=np.float32)
        pF[:, F_B1C:F_B1C + DB] = b1cb
        pF[:, F_H2:F_H2 + SB] = h2.reshape(SB, P).T.astype(np.float32)
        pF[:, F_M2:F_M2 + SB] = (2.0 * mask).reshape(SB, P).T.astype(
            np.float32)

        in_maps.append({"packA": pA, "packB1": pB1,
                        "packI": packI, "packB2": packB2, "packF": pF})
    return in_maps


def kernel(inp, inp_len, aspect, w1, b1, w2, b2):
    from concourse.bass_utils import run_bass_kernel_spmd

    nc = _get_compiled()
    in_maps = _host_prep(inp, inp_len, aspect, w1, b1, w2, b2)
    res = run_bass_kernel_spmd(nc, in_maps, core_ids=list(range(N_CORES)))
    # device stores bf16; rows are unit-norm so bf16 rounding adds ~0.2%
    return np.stack([res.results[i]["out"] for i in range(N_CORES)],
                    axis=0).astype(np.float32)

